# revision 11
# baseline (speedup 1.0000x reference)
"""Trainium2 Bass kernel for nn_Decoder (ragged expand + 4-layer decoder).

Sharding: 8 cores = 4 pairs. Pair p handles batch b=p (data parallel over B).
Within a pair, Megatron TP-2: wq/wk/wv/w1/w3 column-sharded, wo/w2 row-sharded,
pair AllReduce after attention-out and FFN-out projections.

Device layout: activations live transposed in SBUF as xT[d, l] (partition=d),
tiled [128, n_dtiles, L]. All matmuls bf16 inputs, fp32 PSUM accumulate;
residual stream fp32.
"""

import numpy as np
import ml_dtypes

B, L, D, KB = 4, 1024, 1024, 128
H, HD, NL, FF = 16, 64, 4, 4096
EPS = 1e-5
NCORES = 8
TP = 2
DQ = D // TP      # 512 local q/k/v dims (8 heads)
DF = FF // TP     # 2048 local ffn dims
NH = H // TP      # 8 local heads
CH = 512          # sequence chunk for moving dim
NCH = L // CH     # 2
NDT = D // 128    # 8 d-tiles
NQT = DQ // 128   # 4 local qkv tiles
NFT = DF // 128   # 16 local ffn tiles
NLT = L // 128    # 8 sequence tiles

_CACHE = {}


def _build(pairs=4):
    import concourse.mybir as mybir
    import concourse.tile as tile
    from concourse import bacc

    f32 = mybir.dt.float32
    bf = mybir.dt.bfloat16
    Alu = mybir.AluOpType
    Act = mybir.ActivationFunctionType

    nc = bacc.Bacc(
        "TRN2", target_bir_lowering=False, debug=False, num_devices=2 * pairs
    )

    # ---- DRAM I/O (per core) ----
    xp_d = nc.dram_tensor("xp", [KB, D], f32, kind="ExternalInput")
    bnd_d = nc.dram_tensor("bnd", [KB, 1], f32, kind="ExternalInput")
    bnds_d = nc.dram_tensor("bnds", [KB, 1], f32, kind="ExternalInput")
    xres_d = nc.dram_tensor("xres", [128, NDT, L], f32, kind="ExternalInput")
    cos_d = nc.dram_tensor("cosr", [128, L], bf, kind="ExternalInput")
    sin_d = nc.dram_tensor("sinr", [128, L], bf, kind="ExternalInput")
    wq_d = nc.dram_tensor("wq", [NL, 128, NDT, DQ], bf, kind="ExternalInput")
    wk_d = nc.dram_tensor("wk", [NL, 128, NDT, DQ], bf, kind="ExternalInput")
    wv_d = nc.dram_tensor("wv", [NL, 128, NDT, DQ], bf, kind="ExternalInput")
    wo_d = nc.dram_tensor("wo", [NL, 128, NQT, D], bf, kind="ExternalInput")
    w1_d = nc.dram_tensor("w1", [NL, 128, NDT, DF], bf, kind="ExternalInput")
    w3_d = nc.dram_tensor("w3", [NL, 128, NDT, DF], bf, kind="ExternalInput")
    w2_d = nc.dram_tensor("w2", [NL, 128, NFT, D], bf, kind="ExternalInput")
    onec_d = nc.dram_tensor("onec", [128, 1], bf, kind="ExternalInput")
    eps_d = nc.dram_tensor("epsc", [1, 1], f32, kind="ExternalInput")
    oner_d = nc.dram_tensor("oner", [1, 128], bf, kind="ExternalInput")
    nwr_d = nc.dram_tensor("nwr", [1, D], bf, kind="ExternalInput")
    rsg_d = nc.dram_tensor("rsg", [128, 128], bf, kind="ExternalInput")
    out_d = nc.dram_tensor("out", [D, L], f32, kind="ExternalOutput")

    groups = [[2 * p, 2 * p + 1] for p in range(pairs)]

    with tile.TileContext(nc) as tc:
        with (
            tc.tile_pool(name="persist", bufs=1) as pp,
            tc.tile_pool(name="gbig", bufs=1) as gp,
            tc.tile_pool(name="wres", bufs=1) as wp,
            tc.tile_pool(name="wstream", bufs=2) as ws,
            tc.tile_pool(name="small", bufs=4) as sp,
            tc.tile_pool(name="rows", bufs=2) as rp,
            tc.tile_pool(name="stage", bufs=2) as st,
            tc.tile_pool(name="psA", bufs=2, space="PSUM") as psA,
            tc.tile_pool(name="psB", bufs=2, space="PSUM") as psB,
            tc.tile_pool(name="psC", bufs=2, space="PSUM") as psC,
            tc.tile_pool(name="psD", bufs=1, space="PSUM") as psD,
            tc.tile_pool(name="psE", bufs=1, space="PSUM") as psE,
            tc.tile_pool(name="dram", bufs=2, space="DRAM") as dp,
        ):
            # ---- persistent tiles ----
            xT = pp.tile([128, NDT, L], f32)
            hh = pp.tile([128, NDT, L], bf)
            qT = pp.tile([128, NQT, L], bf)
            kT = pp.tile([128, NQT, L], bf)
            vS = pp.tile([128, NLT, NH * 65], bf)
            oT = pp.tile([128, NQT, L], bf)
            cosr = pp.tile([128, L], bf)
            sinr = pp.tile([128, L], bf)
            onec = pp.tile([128, 1], bf)
            epsc = pp.tile([1, 1], f32)
            oner = pp.tile([1, 128], bf)
            nwr = pp.tile([1, D], bf)
            rsg = pp.tile([128, 128], bf)
            bnd = pp.tile([KB, 1], f32)
            bnds = pp.tile([KB, 1], f32)
            # big scratch region (tag-shared): g for FFN, out staging at end
            g = gp.tile([128, NFT, L], bf, tag="big")

            nc.sync.dma_start(cosr[:], cos_d[:])
            nc.sync.dma_start(sinr[:], sin_d[:])
            nc.sync.dma_start(onec[:], onec_d[:])
            nc.sync.dma_start(epsc[:], eps_d[:])
            nc.sync.dma_start(oner[:], oner_d[:])
            nc.sync.dma_start(nwr[:], nwr_d[:])
            nc.sync.dma_start(rsg[:], rsg_d[:])
            nc.sync.dma_start(bnd[:], bnd_d[:])
            nc.sync.dma_start(bnds[:], bnds_d[:])
            nc.sync.dma_start(xT[:], xres_d[:])

            vS_r = vS.rearrange("p t (h e) -> p t h e", e=65)
            nc.gpsimd.memset(vS_r[:, :, :, 64:65], 1.0)

            # ---- ragged expand: xT += one_hot_gather(x_processed) ----
            # setup scratch shares the FFN g region (disjoint lifetimes)
            setup = gp.tile([128, 4, L], f32, tag="big")
            iota, cmp0, gt, xp = (setup[:, i, :] for i in range(4))
            nc.gpsimd.iota(
                iota, pattern=[[1, L]], base=0, channel_multiplier=0,
                allow_small_or_imprecise_dtypes=True,
            )
            nc.vector.tensor_scalar(cmp0, iota, bnd[:], None, Alu.is_ge)
            # gt = (l >= b_j) - (l >= b_{j+1}), computed via two tensor_scalar
            nc.vector.tensor_scalar(gt, iota, bnds[:], None, Alu.is_ge)
            nc.vector.tensor_sub(gt, cmp0, gt)
            nc.sync.dma_start(xp, xp_d[:])
            for dt in range(NDT):
                for ch in range(NCH):
                    cs = slice(ch * CH, (ch + 1) * CH)
                    ps = psA.tile([128, CH], f32, tag="A")
                    nc.tensor.matmul(
                        ps[:], xp[:, dt * 128:(dt + 1) * 128], gt[:, cs],
                        start=True, stop=True,
                    )
                    nc.vector.tensor_add(xT[:, dt, cs], xT[:, dt, cs], ps[:])

            # ---- helpers ----
            def rmsnorm(dest, dest_f32_with_nw=False):
                """h = xT * rsqrt(mean(xT^2)+eps) [* norm_w], written per chunk."""
                for ch in range(NCH):
                    cs = slice(ch * CH, (ch + 1) * CH)
                    ssp = psD.tile([1, CH], f32, tag="D")
                    for dt in range(NDT):
                        sq = sp.tile([128, CH], bf, tag="sq")
                        nc.scalar.activation(sq[:], xT[:, dt, cs], Act.Square)
                        nc.tensor.matmul(
                            ssp[:], onec[:], sq[:],
                            start=(dt == 0), stop=(dt == NDT - 1),
                        )
                    srow = rp.tile([1, CH], f32, tag="srow")
                    nc.scalar.activation(
                        srow[:], ssp[:], Act.Sqrt, bias=epsc[:], scale=1.0 / D
                    )
                    rrow = rp.tile([1, CH], bf, tag="rrow")
                    with nc.allow_low_precision(reason="bf16 norm scale"):
                        nc.vector.reciprocal(rrow[:], srow[:])
                    if not dest_f32_with_nw:
                        bps = psE.tile([128, CH], f32, tag="E")
                        nc.tensor.matmul(
                            bps[:], oner[:], rrow[:], start=True, stop=True
                        )
                        for dt in range(NDT):
                            nc.vector.tensor_mul(
                                dest[:, dt, cs], xT[:, dt, cs], bps[:]
                            )
                    else:
                        for dt in range(NDT):
                            bps = psE.tile([128, CH], f32, tag="E")
                            nc.tensor.matmul(
                                bps[:], nwr[:, dt * 128:(dt + 1) * 128],
                                rrow[:], start=True, stop=True,
                            )
                            nc.vector.tensor_mul(
                                dest[:, dt, cs], xT[:, dt, cs], bps[:]
                            )

            def allreduce_add(get_psum, tag):
                """Project-out + pair allreduce + add into xT, per chunk.

                get_psum(ot, ch) -> psum tile [128, CH] (projection output)."""
                for ch in range(NCH):
                    cs = slice(ch * CH, (ch + 1) * CH)
                    inb = dp.tile([128, NDT, CH], f32, tag=f"{tag}i")
                    outb = dp.tile([128, NDT, CH], f32, tag=f"{tag}o")
                    for ot in range(NDT):
                        ps = get_psum(ot, ch)
                        stg = st.tile([128, CH], f32, tag="stg")
                        nc.scalar.activation(stg[:], ps[:], Act.Copy)
                        nc.gpsimd.dma_start(inb[:, ot, :], stg[:])
                    nc.gpsimd.collective_compute(
                        "AllReduce", Alu.add, replica_groups=groups,
                        ins=[inb.opt()], outs=[outb.opt()],
                    )
                    for ot in range(NDT):
                        ret = st.tile([128, CH], f32, tag="ret")
                        nc.gpsimd.dma_start(ret[:], outb[:, ot, :])
                        nc.vector.tensor_add(xT[:, ot, cs], xT[:, ot, cs], ret[:])

            # ==================== layers ====================
            for ly in range(NL):
                # ---- attention ----
                rmsnorm(hh)
                wq = wp.tile([128, NDT, DQ], bf, tag="wq")
                wk = wp.tile([128, NDT, DQ], bf, tag="wk")
                wv = wp.tile([128, NDT, DQ], bf, tag="wv")
                wo = wp.tile([128, NQT, D], bf, tag="wo")
                nc.sync.dma_start(wq[:], wq_d[ly])
                nc.sync.dma_start(wk[:], wk_d[ly])
                nc.sync.dma_start(wv[:], wv_d[ly])
                nc.sync.dma_start(wo[:], wo_d[ly])

                # q/k projections + rope
                for wt, dest in ((wq, qT), (wk, kT)):
                    for ot in range(NQT):
                        for ch in range(NCH):
                            cs = slice(ch * CH, (ch + 1) * CH)
                            ps = psA.tile([128, CH], f32, tag="A")
                            for dt in range(NDT):
                                nc.tensor.matmul(
                                    ps[:],
                                    wt[:, dt, ot * 128:(ot + 1) * 128],
                                    hh[:, dt, cs],
                                    start=(dt == 0), stop=(dt == NDT - 1),
                                )
                            qsb = sp.tile([128, CH], bf, tag="qsb")
                            nc.scalar.activation(qsb[:], ps[:], Act.Copy)
                            t_t = sp.tile([128, CH], bf, tag="ropet")
                            u_t = sp.tile([128, CH], bf, tag="ropeu")
                            nc.vector.tensor_mul(t_t[:], qsb[:], cosr[:, cs])
                            nc.vector.tensor_mul(u_t[:], qsb[:], sinr[:, cs])
                            # rotate-half with sign via permutation matmul
                            us = psB.tile([128, CH], f32, tag="B")
                            nc.tensor.matmul(
                                us[:], rsg[:], u_t[:], start=True, stop=True
                            )
                            nc.vector.tensor_add(
                                dest[:, ot, cs], t_t[:], us[:]
                            )

                # v projection, non-transposed [l, dv], with ones column kept
                for lt in range(NLT):
                    ls = slice(lt * 128, (lt + 1) * 128)
                    ps = psA.tile([128, DQ], f32, tag="A")
                    for dt in range(NDT):
                        nc.tensor.matmul(
                            ps[:], hh[:, dt, ls], wv[:, dt, :],
                            start=(dt == 0), stop=(dt == NDT - 1),
                        )
                    nc.scalar.activation(
                        vS_r[:, lt, :, 0:64],
                        ps[:].rearrange("p (h e) -> p h e", e=64),
                        Act.Copy,
                    )

                # attention per (head, chunk): sT scores -> exp/mask -> o accum
                for h8 in range(NH):
                    pb = h8 // 2
                    po = (h8 % 2) * 64
                    for ch in range(NCH):
                        cs = slice(ch * CH, (ch + 1) * CH)
                        nlts = 4 if ch == 0 else 8
                        pot = psC.tile([65, CH], f32, tag="C")
                        for lt in range(nlts):
                            ss = psB.tile([128, CH], f32, tag="B")
                            nc.tensor.matmul(
                                ss[:],
                                kT[po:po + 64, pb, lt * 128:(lt + 1) * 128],
                                qT[po:po + 64, pb, cs],
                                start=True, stop=True,
                            )
                            at = sp.tile([128, CH], bf, tag="at")
                            d0 = 128 * lt - CH * ch
                            v0 = max(0, d0)
                            if v0 > 0:
                                nc.gpsimd.memset(at[:, 0:v0], 0.0)
                            nc.scalar.activation(
                                at[:, v0:CH], ss[:, v0:CH], Act.Exp, scale=0.125
                            )
                            if 0 <= d0 < CH:
                                nc.gpsimd.affine_select(
                                    at[:, d0:d0 + 128], at[:, d0:d0 + 128],
                                    pattern=[[1, 128]],
                                    compare_op=Alu.is_ge,
                                    fill=0.0, base=0, channel_multiplier=-1,
                                )
                            nc.tensor.matmul(
                                pot[:], vS[:, lt, h8 * 65:(h8 + 1) * 65], at[:],
                                start=(lt == 0), stop=(lt == nlts - 1),
                            )
                        rc = rp.tile([1, CH], bf, tag="rc")
                        with nc.allow_low_precision(reason="bf16 softmax scale"):
                            nc.vector.reciprocal(rc[:], pot[64:65, :])
                        bps = psE.tile([128, CH], f32, tag="E")
                        nc.tensor.matmul(
                            bps[0:64, :], oner[:, 0:64], rc[:],
                            start=True, stop=True,
                        )
                        bsb = sp.tile([64, CH], bf, tag="bsb")
                        nc.scalar.activation(bsb[:], bps[0:64, :], Act.Copy)
                        nc.vector.tensor_mul(
                            oT[po:po + 64, pb, cs], pot[0:64, :], bsb[:]
                        )

                # wo projection + allreduce + residual
                wo_ps = {}

                def wo_psum(ot, ch, _wo=wo, _wo_ps=wo_ps):
                    cs = slice(ch * CH, (ch + 1) * CH)
                    ps = psA.tile([128, CH], f32, tag="A")
                    for kt in range(NQT):
                        nc.tensor.matmul(
                            ps[:], _wo[:, kt, ot * 128:(ot + 1) * 128],
                            oT[:, kt, cs],
                            start=(kt == 0), stop=(kt == NQT - 1),
                        )
                    return ps

                allreduce_add(wo_psum, f"at{ly}")

                # ---- FFN ----
                rmsnorm(hh)
                for ch in range(NCH):
                    cs = slice(ch * CH, (ch + 1) * CH)
                    # w1 -> silu -> g
                    for ftb in range(4):
                        pss = []
                        for j in range(4):
                            tagp = ("A", "A", "B", "B")[j]
                            pool = (psA, psA, psB, psB)[j]
                            pss.append(pool.tile(
                                [128, CH], f32, tag=tagp, name=f"ps1_{j}"
                            ))
                        for dt in range(NDT):
                            w1s = ws.tile([128, 512], bf, tag="w1s")
                            nc.sync.dma_start(
                                w1s[:], w1_d[ly, :, dt, ftb * 512:(ftb + 1) * 512]
                            )
                            for j in range(4):
                                nc.tensor.matmul(
                                    pss[j][:],
                                    w1s[:, j * 128:(j + 1) * 128],
                                    hh[:, dt, cs],
                                    start=(dt == 0), stop=(dt == NDT - 1),
                                )
                        for j in range(4):
                            nc.scalar.activation(
                                g[:, ftb * 4 + j, cs], pss[j][:], Act.Silu
                            )
                    # w3 -> u = g * w3 (in place in g)
                    for ftb in range(4):
                        pss = []
                        for j in range(4):
                            tagp = ("A", "A", "B", "B")[j]
                            pool = (psA, psA, psB, psB)[j]
                            pss.append(pool.tile(
                                [128, CH], f32, tag=tagp, name=f"ps3_{j}"
                            ))
                        for dt in range(NDT):
                            w3s = ws.tile([128, 512], bf, tag="w3s")
                            nc.sync.dma_start(
                                w3s[:], w3_d[ly, :, dt, ftb * 512:(ftb + 1) * 512]
                            )
                            for j in range(4):
                                nc.tensor.matmul(
                                    pss[j][:],
                                    w3s[:, j * 128:(j + 1) * 128],
                                    hh[:, dt, cs],
                                    start=(dt == 0), stop=(dt == NDT - 1),
                                )
                        for j in range(4):
                            ft = ftb * 4 + j
                            nc.vector.tensor_mul(
                                g[:, ft, cs], g[:, ft, cs], pss[j][:]
                            )
                    # w2: 8 concurrent psums over all banks
                    pools8 = (psA, psA, psB, psB, psC, psC, psD, psE)
                    tags8 = ("A", "A", "B", "B", "C", "C", "D", "E")
                    pso = [
                        pools8[ot].tile(
                            [128, CH], f32, tag=tags8[ot], name=f"pso_{ot}"
                        )
                        for ot in range(NDT)
                    ]
                    for ft in range(NFT):
                        w2s = ws.tile([128, D], bf, tag="w2s")
                        nc.sync.dma_start(w2s[:], w2_d[ly, :, ft, :])
                        for ot in range(NDT):
                            nc.tensor.matmul(
                                pso[ot][:],
                                w2s[:, ot * 128:(ot + 1) * 128],
                                g[:, ft, cs],
                                start=(ft == 0), stop=(ft == NFT - 1),
                            )
                    # allreduce this chunk + residual
                    inb = dp.tile([128, NDT, CH], f32, tag=f"ff{ly}i")
                    outb = dp.tile([128, NDT, CH], f32, tag=f"ff{ly}o")
                    for ot in range(NDT):
                        stg = st.tile([128, CH], f32, tag="stg")
                        nc.scalar.activation(stg[:], pso[ot][:], Act.Copy)
                        nc.gpsimd.dma_start(inb[:, ot, :], stg[:])
                    nc.gpsimd.collective_compute(
                        "AllReduce", Alu.add, replica_groups=groups,
                        ins=[inb.opt()], outs=[outb.opt()],
                    )
                    for ot in range(NDT):
                        ret = st.tile([128, CH], f32, tag="ret")
                        nc.gpsimd.dma_start(ret[:], outb[:, ot, :])
                        nc.vector.tensor_add(
                            xT[:, ot, cs], xT[:, ot, cs], ret[:]
                        )

            # ---- final norm (* norm_w) and output ----
            outsb = gp.tile([128, NDT, L], f32, tag="big")
            rmsnorm(outsb, dest_f32_with_nw=True)
            nc.sync.dma_start(out_d[:].rearrange("(dt p) l -> p dt l", p=128), outsb[:])

    nc.finalize()
    return nc


def _get_nc(pairs=4):
    if pairs not in _CACHE:
        _CACHE[pairs] = _build(pairs)
    return _CACHE[pairs]


def _rsign_const():
    # usigned = R2 @ u : usigned[0:32] = -u[32:64], usigned[32:64] = u[0:32]
    # per 64-block (two heads per 128-partition tile). Pass lhsT = R2.T.
    rh = np.zeros((64, 64), np.float32)
    rh[np.arange(32), np.arange(32) + 32] = -1.0
    rh[np.arange(32) + 32, np.arange(32)] = 1.0
    r2 = np.zeros((128, 128), np.float32)
    r2[0:64, 0:64] = rh
    r2[64:128, 64:128] = rh
    return np.ascontiguousarray(r2.T).astype(ml_dtypes.bfloat16)


def _prep_core_inputs(inputs, b, t):
    """Host-side shard/layout prep for core (pair b, tp half t)."""
    f32 = np.float32
    bf = ml_dtypes.bfloat16
    x_processed = np.asarray(inputs["x_processed"], f32)
    boundaries = np.asarray(inputs["boundaries"], np.int32)
    x_residual = np.asarray(inputs["x_residual"], f32)
    cos = np.asarray(inputs["cos"], f32)
    sin = np.asarray(inputs["sin"], f32)
    wq = np.asarray(inputs["wq"], f32)
    wk = np.asarray(inputs["wk"], f32)
    wv = np.asarray(inputs["wv"], f32)
    wo = np.asarray(inputs["wo"], f32)
    w1 = np.asarray(inputs["w1"], f32)
    w2 = np.asarray(inputs["w2"], f32)
    w3 = np.asarray(inputs["w3"], f32)
    attn_norm_w = np.asarray(inputs["attn_norm_w"], f32)
    ffn_norm_w = np.asarray(inputs["ffn_norm_w"], f32)
    norm_w = np.asarray(inputs["norm_w"], f32)

    bnd = boundaries[b].astype(f32).copy()
    bnd[0] = 0.0  # match searchsorted-then-clip(>=0) semantics
    bnds = np.concatenate([bnd[1:], [np.float32(2 * L)]])

    def dtile(w, ncols):  # [D, ncols] -> [128, D//128, ncols]
        return np.ascontiguousarray(
            w.reshape(-1, 128, ncols).transpose(1, 0, 2)
        )

    qs = slice(t * DQ, (t + 1) * DQ)
    fs = slice(t * DF, (t + 1) * DF)
    wq_s = np.stack([
        dtile((attn_norm_w[l][:, None] * wq[l])[:, qs].astype(bf), DQ)
        for l in range(NL)
    ])
    wk_s = np.stack([
        dtile((attn_norm_w[l][:, None] * wk[l])[:, qs].astype(bf), DQ)
        for l in range(NL)
    ])
    wv_s = np.stack([
        dtile((attn_norm_w[l][:, None] * wv[l])[:, qs].astype(bf), DQ)
        for l in range(NL)
    ])
    wo_s = np.stack([dtile(wo[l][qs, :].astype(bf), D) for l in range(NL)])
    w1_s = np.stack([
        dtile((ffn_norm_w[l][:, None] * w1[l])[:, fs].astype(bf), DF)
        for l in range(NL)
    ])
    w3_s = np.stack([
        dtile((ffn_norm_w[l][:, None] * w3[l])[:, fs].astype(bf), DF)
        for l in range(NL)
    ])
    w2_s = np.stack([dtile(w2[l][fs, :].astype(bf), D) for l in range(NL)])

    cosT = cos.T.astype(bf)  # [HD, L]
    sinT = sin.T.astype(bf)
    cos_rep = np.concatenate([cosT, cosT], axis=0)  # [128, L]
    sin_rep = np.concatenate([sinT, sinT], axis=0)

    xres_t = np.ascontiguousarray(
        x_residual[b].T.reshape(NDT, 128, L).transpose(1, 0, 2)
    )

    return {
        "xp": np.ascontiguousarray(x_processed[b]),
        "bnd": bnd[:, None],
        "bnds": bnds[:, None],
        "xres": xres_t,
        "cosr": np.ascontiguousarray(cos_rep),
        "sinr": np.ascontiguousarray(sin_rep),
        "wq": wq_s, "wk": wk_s, "wv": wv_s, "wo": wo_s,
        "w1": w1_s, "w3": w3_s, "w2": w2_s,
        "onec": np.ones((128, 1), bf),
        "rsg": _rsign_const(),
        "epsc": np.full((1, 1), EPS, f32),
        "oner": np.ones((1, 128), bf),
        "nwr": norm_w[None, :].astype(bf),
    }


def kernel(**inputs) -> np.ndarray:
    from concourse.bass_utils import run_bass_kernel_spmd

    nc = _get_nc(4)
    in_maps = []
    for c in range(NCORES):
        in_maps.append(_prep_core_inputs(inputs, c // 2, c % 2))
    res = run_bass_kernel_spmd(nc, in_maps, list(range(NCORES)))
    out = np.empty((B, L, D), np.float32)
    for b in range(B):
        out[b] = res.results[2 * b]["out"].T
    return out


# revision 15
# speedup vs baseline: 1.0883x; 1.0883x over previous
"""Trainium2 Bass kernel for nn_Decoder (ragged expand + 4-layer decoder).

Sharding: 8 cores = 4 pairs. Pair p handles batch b=p (data parallel over B).
Within a pair, Megatron TP-2: wq/wk/wv/w1/w3 column-sharded, wo/w2 row-sharded,
pair AllReduce after attention-out and FFN-out projections.

Device layout: activations live transposed in SBUF as xT[d, l] (partition=d),
tiled [128, n_dtiles, L]. All matmuls bf16 inputs, fp32 PSUM accumulate;
residual stream fp32.
"""

import numpy as np
import ml_dtypes

B, L, D, KB = 4, 1024, 1024, 128
H, HD, NL, FF = 16, 64, 4, 4096
EPS = 1e-5
NCORES = 8
TP = 2
DQ = D // TP      # 512 local q/k/v dims (8 heads)
DF = FF // TP     # 2048 local ffn dims
NH = H // TP      # 8 local heads
CH = 512          # sequence chunk for moving dim
NCH = L // CH     # 2
NDT = D // 128    # 8 d-tiles
NQT = DQ // 128   # 4 local qkv tiles
NFT = DF // 128   # 16 local ffn tiles
NLT = L // 128    # 8 sequence tiles

_CACHE = {}
# debug/bisect toggles (consulted at build time)
OPTS = {"recip_approx": True, "cc_shared": False, "ffn_cc_bf16": True}


def _build(pairs=4):
    import concourse.mybir as mybir
    import concourse.tile as tile
    from concourse import bacc

    f32 = mybir.dt.float32
    bf = mybir.dt.bfloat16
    Alu = mybir.AluOpType
    Act = mybir.ActivationFunctionType

    nc = bacc.Bacc(
        "TRN2", target_bir_lowering=False, debug=False, num_devices=2 * pairs
    )
    sim_mode = pairs == 1  # CoreSim lacks Silu; decompose as x*sigmoid(x)

    # ---- DRAM I/O (per core) ----
    xp_d = nc.dram_tensor("xp", [KB, D], f32, kind="ExternalInput")
    bnd_d = nc.dram_tensor("bnd", [KB, 1], f32, kind="ExternalInput")
    bnds_d = nc.dram_tensor("bnds", [KB, 1], f32, kind="ExternalInput")
    xres_d = nc.dram_tensor("xres", [128, NDT, L], f32, kind="ExternalInput")
    cos_d = nc.dram_tensor("cosr", [128, L], bf, kind="ExternalInput")
    sin_d = nc.dram_tensor("sinr", [128, L], bf, kind="ExternalInput")
    wq_d = nc.dram_tensor("wq", [NL, 128, NDT, DQ], bf, kind="ExternalInput")
    wk_d = nc.dram_tensor("wk", [NL, 128, NDT, DQ], bf, kind="ExternalInput")
    wv_d = nc.dram_tensor("wv", [NL, 128, NDT, DQ], bf, kind="ExternalInput")
    wo_d = nc.dram_tensor("wo", [NL, 128, NDT, D], bf, kind="ExternalInput")
    w1_d = nc.dram_tensor("w1", [NL, 128, NDT, DF], bf, kind="ExternalInput")
    w3_d = nc.dram_tensor("w3", [NL, 128, NDT, DF], bf, kind="ExternalInput")
    w2_d = nc.dram_tensor("w2", [NL, 128, NFT, D], bf, kind="ExternalInput")
    onec_d = nc.dram_tensor("onec", [128, 1], bf, kind="ExternalInput")
    eps_d = nc.dram_tensor("epsc", [1, 1], f32, kind="ExternalInput")
    oner_d = nc.dram_tensor("oner", [1, 128], bf, kind="ExternalInput")
    nwr_d = nc.dram_tensor("nwr", [1, D], bf, kind="ExternalInput")
    rsg_d = nc.dram_tensor("rsg", [128, 128], bf, kind="ExternalInput")
    out_d = nc.dram_tensor("out", [D, L], f32, kind="ExternalOutput")

    groups = [[2 * p, 2 * p + 1] for p in range(pairs)]

    with tile.TileContext(nc) as tc:
        with (
            tc.tile_pool(name="persist", bufs=1) as pp,
            tc.tile_pool(name="gbig", bufs=1) as gp,
            tc.tile_pool(name="wres", bufs=1) as wp,
            tc.tile_pool(name="wstream", bufs=2) as ws,
            tc.tile_pool(name="small", bufs=3) as sp,
            tc.tile_pool(name="rows", bufs=2) as rp,
            tc.tile_pool(name="stage", bufs=2) as st,
            tc.tile_pool(name="psA", bufs=2, space="PSUM") as psA,
            tc.tile_pool(name="psB", bufs=2, space="PSUM") as psB,
            tc.tile_pool(name="psC", bufs=2, space="PSUM") as psC,
            tc.tile_pool(name="psD", bufs=1, space="PSUM") as psD,
            tc.tile_pool(name="psE", bufs=1, space="PSUM") as psE,
            tc.tile_pool(name="dram", bufs=2, space="DRAM") as dp,
        ):
            # ---- persistent tiles ----
            xT = pp.tile([128, NDT, L], f32)
            hh = pp.tile([128, NDT, L], bf)
            qT = pp.tile([128, NQT, L], bf)
            kT = pp.tile([128, NQT, L], bf)
            vS = pp.tile([128, NLT, NH * 65], bf)
            oT = pp.tile([128, NQT, L], bf)
            cosr = pp.tile([128, L], bf)
            sinr = pp.tile([128, L], bf)
            onec = pp.tile([128, 1], bf)
            epsc = pp.tile([1, 1], f32)
            oner = pp.tile([1, 128], bf)
            nwr = pp.tile([1, D], bf)
            rsg = pp.tile([128, 128], bf)
            bnd = pp.tile([KB, 1], f32)
            bnds = pp.tile([KB, 1], f32)
            # big scratch region (tag-shared): g for FFN, out staging at end
            g = gp.tile([128, NFT, L], bf, tag="big")

            nc.sync.dma_start(cosr[:], cos_d[:])
            nc.sync.dma_start(sinr[:], sin_d[:])
            nc.sync.dma_start(onec[:], onec_d[:])
            nc.sync.dma_start(epsc[:], eps_d[:])
            nc.sync.dma_start(oner[:], oner_d[:])
            nc.sync.dma_start(nwr[:], nwr_d[:])
            nc.sync.dma_start(rsg[:], rsg_d[:])
            nc.sync.dma_start(bnd[:], bnd_d[:])
            nc.sync.dma_start(bnds[:], bnds_d[:])
            nc.sync.dma_start(xT[:], xres_d[:])

            vS_r = vS.rearrange("p t (h e) -> p t h e", e=65)
            nc.gpsimd.memset(vS_r[:, :, :, 64:65], 1.0)

            # ---- ragged expand: xT += one_hot_gather(x_processed) ----
            # setup scratch shares the FFN g region (disjoint lifetimes)
            setup = gp.tile([128, 4, L], f32, tag="big")
            iota, cmp0, gt, xp = (setup[:, i, :] for i in range(4))
            nc.gpsimd.iota(
                iota, pattern=[[1, L]], base=0, channel_multiplier=0,
                allow_small_or_imprecise_dtypes=True,
            )
            nc.vector.tensor_scalar(cmp0, iota, bnd[:], None, Alu.is_ge)
            # gt = (l >= b_j) - (l >= b_{j+1}), computed via two tensor_scalar
            nc.vector.tensor_scalar(gt, iota, bnds[:], None, Alu.is_ge)
            nc.vector.tensor_sub(gt, cmp0, gt)
            nc.sync.dma_start(xp, xp_d[:])
            for dt in range(NDT):
                for ch in range(NCH):
                    cs = slice(ch * CH, (ch + 1) * CH)
                    ps = psA.tile([128, CH], f32, tag="A")
                    nc.tensor.matmul(
                        ps[:], xp[:, dt * 128:(dt + 1) * 128], gt[:, cs],
                        start=True, stop=True,
                    )
                    nc.vector.tensor_add(xT[:, dt, cs], xT[:, dt, cs], ps[:])

            # ---- helpers ----
            def rmsnorm(dest, dest_f32_with_nw=False):
                """h = xT * rsqrt(mean(xT^2)+eps) [* norm_w], written per chunk."""
                for ch in range(NCH):
                    cs = slice(ch * CH, (ch + 1) * CH)
                    ssp = psD.tile([1, CH], f32, tag="D")
                    for dt in range(NDT):
                        sq = sp.tile([128, CH], bf, tag="sc3")
                        nc.scalar.activation(sq[:], xT[:, dt, cs], Act.Square)
                        nc.tensor.matmul(
                            ssp[:], onec[:], sq[:],
                            start=(dt == 0), stop=(dt == NDT - 1),
                        )
                    srow = rp.tile([1, CH], f32, tag="srow")
                    nc.scalar.activation(
                        srow[:], ssp[:], Act.Sqrt, bias=epsc[:], scale=1.0 / D
                    )
                    rf32 = rp.tile([1, CH], f32, tag="rf32")
                    if OPTS["recip_approx"]:
                        nc.vector.reciprocal_approx_fast(rf32[:], srow[:])
                    else:
                        nc.vector.reciprocal(rf32[:], srow[:])
                    rrow = rp.tile([1, CH], bf, tag="rbf")
                    nc.scalar.activation(rrow[:], rf32[:], Act.Copy)
                    if not dest_f32_with_nw:
                        bps = psE.tile([128, CH], f32, tag="E")
                        nc.tensor.matmul(
                            bps[:], oner[:], rrow[:], start=True, stop=True
                        )
                        for dt in range(NDT):
                            nc.vector.tensor_mul(
                                dest[:, dt, cs], xT[:, dt, cs], bps[:]
                            )
                    else:
                        for dt in range(NDT):
                            bps = psE.tile([128, CH], f32, tag="E")
                            nc.tensor.matmul(
                                bps[:], nwr[:, dt * 128:(dt + 1) * 128],
                                rrow[:], start=True, stop=True,
                            )
                            nc.vector.tensor_mul(
                                dest[:, dt, cs], xT[:, dt, cs], bps[:]
                            )

            # ==================== layers ====================
            for ly in range(NL):
                # ---- attention ----
                rmsnorm(hh)
                wq = wp.tile([128, NDT, DQ], bf, tag="wq")
                wk = wp.tile([128, NDT, DQ], bf, tag="wk")
                wv = wp.tile([128, NDT, DQ], bf, tag="wv")
                wo = wp.tile([128, NDT, D], bf, tag="wo")
                nc.sync.dma_start(wq[:], wq_d[ly])
                nc.sync.dma_start(wk[:], wk_d[ly])
                nc.sync.dma_start(wv[:], wv_d[ly])
                nc.sync.dma_start(wo[:], wo_d[ly])

                # q/k projections + rope
                for wt, dest in ((wq, qT), (wk, kT)):
                    for ot in range(NQT):
                        for ch in range(NCH):
                            cs = slice(ch * CH, (ch + 1) * CH)
                            ps = psA.tile([128, CH], f32, tag="A")
                            for dt in range(NDT):
                                nc.tensor.matmul(
                                    ps[:],
                                    wt[:, dt, ot * 128:(ot + 1) * 128],
                                    hh[:, dt, cs],
                                    start=(dt == 0), stop=(dt == NDT - 1),
                                )
                            qsb = sp.tile([128, CH], bf, tag="sc1")
                            nc.scalar.activation(qsb[:], ps[:], Act.Copy)
                            t_t = sp.tile([128, CH], bf, tag="sc2")
                            u_t = sp.tile([128, CH], bf, tag="sc3")
                            nc.vector.tensor_mul(t_t[:], qsb[:], cosr[:, cs])
                            nc.vector.tensor_mul(u_t[:], qsb[:], sinr[:, cs])
                            # rotate-half with sign via permutation matmul
                            us = psB.tile([128, CH], f32, tag="B")
                            nc.tensor.matmul(
                                us[:], rsg[:], u_t[:], start=True, stop=True
                            )
                            nc.vector.tensor_add(
                                dest[:, ot, cs], t_t[:], us[:]
                            )

                # v projection, non-transposed [l, dv], with ones column kept
                for lt in range(NLT):
                    ls = slice(lt * 128, (lt + 1) * 128)
                    ps = psA.tile([128, DQ], f32, tag="A")
                    for dt in range(NDT):
                        nc.tensor.matmul(
                            ps[:], hh[:, dt, ls], wv[:, dt, :],
                            start=(dt == 0), stop=(dt == NDT - 1),
                        )
                    nc.scalar.activation(
                        vS_r[:, lt, :, 0:64],
                        ps[:].rearrange("p (h e) -> p h e", e=64),
                        Act.Copy,
                    )

                # attention per (head, chunk): sT scores -> exp/mask -> o accum
                for h8 in range(NH):
                    pb = h8 // 2
                    po = (h8 % 2) * 64
                    for ch in range(NCH):
                        cs = slice(ch * CH, (ch + 1) * CH)
                        nlts = 4 if ch == 0 else 8
                        pot = psC.tile([65, CH], f32, tag="C")
                        for lt in range(nlts):
                            ss = psB.tile([128, CH], f32, tag="B")
                            nc.tensor.matmul(
                                ss[:],
                                kT[po:po + 64, pb, lt * 128:(lt + 1) * 128],
                                qT[po:po + 64, pb, cs],
                                start=True, stop=True,
                            )
                            at = sp.tile([128, CH], bf, tag="sc1")
                            d0 = 128 * lt - CH * ch
                            v0 = max(0, d0)
                            if v0 > 0:
                                nc.gpsimd.memset(at[:, 0:v0], 0.0)
                            nc.scalar.activation(
                                at[:, v0:CH], ss[:, v0:CH], Act.Exp, scale=0.125
                            )
                            if 0 <= d0 < CH:
                                nc.gpsimd.affine_select(
                                    at[:, d0:d0 + 128], at[:, d0:d0 + 128],
                                    pattern=[[1, 128]],
                                    compare_op=Alu.is_ge,
                                    fill=0.0, base=0, channel_multiplier=-1,
                                )
                            nc.tensor.matmul(
                                pot[:], vS[:, lt, h8 * 65:(h8 + 1) * 65], at[:],
                                start=(lt == 0), stop=(lt == nlts - 1),
                            )
                        rcf = rp.tile([1, CH], f32, tag="rf32")
                        if OPTS["recip_approx"]:
                            nc.vector.reciprocal_approx_fast(rcf[:], pot[64:65, :])
                        else:
                            nc.vector.reciprocal(rcf[:], pot[64:65, :])
                        rc = rp.tile([1, CH], bf, tag="rbf")
                        nc.scalar.activation(rc[:], rcf[:], Act.Copy)
                        bps = psE.tile([128, CH], f32, tag="E")
                        nc.tensor.matmul(
                            bps[0:64, :], oner[:, 0:64], rc[:],
                            start=True, stop=True,
                        )
                        bsb = sp.tile([64, CH], bf, tag="sc2")
                        nc.scalar.activation(bsb[:], bps[0:64, :], Act.Copy)
                        nc.vector.tensor_mul(
                            oT[po:po + 64, pb, cs], pot[0:64, :], bsb[:]
                        )

                # allgather local oT halves, then full-width wo + residual
                pools8 = (psA, psA, psB, psB, psC, psC, psD, psE)
                tags8 = ("A", "A", "B", "B", "C", "C", "D", "E")
                for ch in range(NCH):
                    cs = slice(ch * CH, (ch + 1) * CH)
                    adsp = "Shared" if OPTS["cc_shared"] else "Local"
                    agi = dp.tile([128, NQT, CH], bf, tag=f"ag{ly}i",
                                  addr_space=adsp)
                    ago = dp.tile([2, 128, NQT, CH], bf, tag=f"ag{ly}o",
                                  addr_space=adsp)
                    nc.gpsimd.dma_start(agi[:], oT[:, :, cs])
                    nc.gpsimd.collective_compute(
                        "AllGather", Alu.bypass, replica_groups=groups,
                        ins=[agi.opt()], outs=[ago.opt()],
                    )
                    pso = [
                        pools8[ot].tile(
                            [128, CH], f32, tag=tags8[ot], name=f"wops_{ot}"
                        )
                        for ot in range(NDT)
                    ]
                    for kt in range(NDT):
                        og = ws.tile([128, CH], bf, tag="og")
                        nc.gpsimd.dma_start(og[:], ago[kt // NQT, :, kt % NQT, :])
                        for ot in range(NDT):
                            nc.tensor.matmul(
                                pso[ot][:],
                                wo[:, kt, ot * 128:(ot + 1) * 128], og[:],
                                start=(kt == 0), stop=(kt == NDT - 1),
                            )
                    for ot in range(NDT):
                        nc.vector.tensor_add(
                            xT[:, ot, cs], xT[:, ot, cs], pso[ot][:]
                        )

                # ---- FFN ----
                rmsnorm(hh)
                for ch in range(NCH):
                    cs = slice(ch * CH, (ch + 1) * CH)
                    # w1 -> silu -> g
                    for ftb in range(4):
                        pss = []
                        for j in range(4):
                            tagp = ("A", "A", "B", "B")[j]
                            pool = (psA, psA, psB, psB)[j]
                            pss.append(pool.tile(
                                [128, CH], f32, tag=tagp, name=f"ps1_{j}"
                            ))
                        for dt in range(NDT):
                            w1s = ws.tile([128, 512], bf, tag="w1s")
                            nc.sync.dma_start(
                                w1s[:], w1_d[ly, :, dt, ftb * 512:(ftb + 1) * 512]
                            )
                            for j in range(4):
                                nc.tensor.matmul(
                                    pss[j][:],
                                    w1s[:, j * 128:(j + 1) * 128],
                                    hh[:, dt, cs],
                                    start=(dt == 0), stop=(dt == NDT - 1),
                                )
                        for j in range(4):
                            if sim_mode:
                                sgt = sp.tile([128, CH], bf, tag="sc2",
                                              name="sgt")
                                nc.scalar.activation(
                                    sgt[:], pss[j][:], Act.Sigmoid
                                )
                                nc.vector.tensor_mul(
                                    g[:, ftb * 4 + j, cs], sgt[:], pss[j][:]
                                )
                            else:
                                nc.scalar.activation(
                                    g[:, ftb * 4 + j, cs], pss[j][:], Act.Silu
                                )
                    # w3 -> u = g * w3 (in place in g)
                    for ftb in range(4):
                        pss = []
                        for j in range(4):
                            tagp = ("A", "A", "B", "B")[j]
                            pool = (psA, psA, psB, psB)[j]
                            pss.append(pool.tile(
                                [128, CH], f32, tag=tagp, name=f"ps3_{j}"
                            ))
                        for dt in range(NDT):
                            w3s = ws.tile([128, 512], bf, tag="w3s")
                            nc.sync.dma_start(
                                w3s[:], w3_d[ly, :, dt, ftb * 512:(ftb + 1) * 512]
                            )
                            for j in range(4):
                                nc.tensor.matmul(
                                    pss[j][:],
                                    w3s[:, j * 128:(j + 1) * 128],
                                    hh[:, dt, cs],
                                    start=(dt == 0), stop=(dt == NDT - 1),
                                )
                        for j in range(4):
                            ft = ftb * 4 + j
                            nc.vector.tensor_mul(
                                g[:, ft, cs], g[:, ft, cs], pss[j][:]
                            )
                    # w2: 8 concurrent psums over all banks
                    pso = [
                        pools8[ot].tile(
                            [128, CH], f32, tag=tags8[ot], name=f"pso_{ot}"
                        )
                        for ot in range(NDT)
                    ]
                    for ft in range(NFT):
                        w2s = ws.tile([128, D], bf, tag="w2s")
                        nc.sync.dma_start(w2s[:], w2_d[ly, :, ft, :])
                        for ot in range(NDT):
                            nc.tensor.matmul(
                                pso[ot][:],
                                w2s[:, ot * 128:(ot + 1) * 128],
                                g[:, ft, cs],
                                start=(ft == 0), stop=(ft == NFT - 1),
                            )
                    # allreduce this chunk + residual
                    ccdt = bf if OPTS["ffn_cc_bf16"] else f32
                    adsp = "Shared" if OPTS["cc_shared"] else "Local"
                    inb = dp.tile([128, NDT, CH], ccdt, tag=f"ff{ly}i",
                                  addr_space=adsp)
                    outb = dp.tile([128, NDT, CH], ccdt, tag=f"ff{ly}o",
                                   addr_space=adsp)
                    for ot in range(NDT):
                        stg = st.tile([128, CH], ccdt, tag="stg")
                        nc.scalar.activation(stg[:], pso[ot][:], Act.Copy)
                        nc.gpsimd.dma_start(inb[:, ot, :], stg[:])
                    nc.gpsimd.collective_compute(
                        "AllReduce", Alu.add, replica_groups=groups,
                        ins=[inb.opt()], outs=[outb.opt()],
                    )
                    for ot in range(NDT):
                        ret = st.tile([128, CH], ccdt, tag="ret")
                        nc.gpsimd.dma_start(ret[:], outb[:, ot, :])
                        nc.vector.tensor_add(
                            xT[:, ot, cs], xT[:, ot, cs], ret[:]
                        )

            # ---- final norm (* norm_w) and output ----
            outsb = gp.tile([128, NDT, L], f32, tag="big")
            rmsnorm(outsb, dest_f32_with_nw=True)
            nc.sync.dma_start(out_d[:].rearrange("(dt p) l -> p dt l", p=128), outsb[:])

    nc.finalize()
    return nc


def _get_nc(pairs=4):
    if pairs not in _CACHE:
        _CACHE[pairs] = _build(pairs)
    return _CACHE[pairs]


def _rsign_const():
    # usigned = R2 @ u : usigned[0:32] = -u[32:64], usigned[32:64] = u[0:32]
    # per 64-block (two heads per 128-partition tile). Pass lhsT = R2.T.
    rh = np.zeros((64, 64), np.float32)
    rh[np.arange(32), np.arange(32) + 32] = -1.0
    rh[np.arange(32) + 32, np.arange(32)] = 1.0
    r2 = np.zeros((128, 128), np.float32)
    r2[0:64, 0:64] = rh
    r2[64:128, 64:128] = rh
    return np.ascontiguousarray(r2.T).astype(ml_dtypes.bfloat16)


def _prep_core_inputs(inputs, b, t):
    """Host-side shard/layout prep for core (pair b, tp half t)."""
    f32 = np.float32
    bf = ml_dtypes.bfloat16
    x_processed = np.asarray(inputs["x_processed"], f32)
    boundaries = np.asarray(inputs["boundaries"], np.int32)
    x_residual = np.asarray(inputs["x_residual"], f32)
    cos = np.asarray(inputs["cos"], f32)
    sin = np.asarray(inputs["sin"], f32)
    wq = np.asarray(inputs["wq"], f32)
    wk = np.asarray(inputs["wk"], f32)
    wv = np.asarray(inputs["wv"], f32)
    wo = np.asarray(inputs["wo"], f32)
    w1 = np.asarray(inputs["w1"], f32)
    w2 = np.asarray(inputs["w2"], f32)
    w3 = np.asarray(inputs["w3"], f32)
    attn_norm_w = np.asarray(inputs["attn_norm_w"], f32)
    ffn_norm_w = np.asarray(inputs["ffn_norm_w"], f32)
    norm_w = np.asarray(inputs["norm_w"], f32)

    bnd = boundaries[b].astype(f32).copy()
    bnd[0] = 0.0  # match searchsorted-then-clip(>=0) semantics
    bnds = np.concatenate([bnd[1:], [np.float32(2 * L)]])

    def dtile(w, ncols):  # [D, ncols] -> [128, D//128, ncols]
        return np.ascontiguousarray(
            w.reshape(-1, 128, ncols).transpose(1, 0, 2)
        )

    qs = slice(t * DQ, (t + 1) * DQ)
    fs = slice(t * DF, (t + 1) * DF)
    wq_s = np.stack([
        dtile((attn_norm_w[l][:, None] * wq[l])[:, qs].astype(bf), DQ)
        for l in range(NL)
    ])
    wk_s = np.stack([
        dtile((attn_norm_w[l][:, None] * wk[l])[:, qs].astype(bf), DQ)
        for l in range(NL)
    ])
    wv_s = np.stack([
        dtile((attn_norm_w[l][:, None] * wv[l])[:, qs].astype(bf), DQ)
        for l in range(NL)
    ])
    wo_s = np.stack([dtile(wo[l].astype(bf), D) for l in range(NL)])
    w1_s = np.stack([
        dtile((ffn_norm_w[l][:, None] * w1[l])[:, fs].astype(bf), DF)
        for l in range(NL)
    ])
    w3_s = np.stack([
        dtile((ffn_norm_w[l][:, None] * w3[l])[:, fs].astype(bf), DF)
        for l in range(NL)
    ])
    w2_s = np.stack([dtile(w2[l][fs, :].astype(bf), D) for l in range(NL)])

    cosT = cos.T.astype(bf)  # [HD, L]
    sinT = sin.T.astype(bf)
    cos_rep = np.concatenate([cosT, cosT], axis=0)  # [128, L]
    sin_rep = np.concatenate([sinT, sinT], axis=0)

    xres_t = np.ascontiguousarray(
        x_residual[b].T.reshape(NDT, 128, L).transpose(1, 0, 2)
    )

    return {
        "xp": np.ascontiguousarray(x_processed[b]),
        "bnd": bnd[:, None],
        "bnds": bnds[:, None],
        "xres": xres_t,
        "cosr": np.ascontiguousarray(cos_rep),
        "sinr": np.ascontiguousarray(sin_rep),
        "wq": wq_s, "wk": wk_s, "wv": wv_s, "wo": wo_s,
        "w1": w1_s, "w3": w3_s, "w2": w2_s,
        "onec": np.ones((128, 1), bf),
        "rsg": _rsign_const(),
        "epsc": np.full((1, 1), EPS, f32),
        "oner": np.ones((1, 128), bf),
        "nwr": norm_w[None, :].astype(bf),
    }


def kernel(**inputs) -> np.ndarray:
    from concourse.bass_utils import run_bass_kernel_spmd

    nc = _get_nc(4)
    in_maps = []
    for c in range(NCORES):
        in_maps.append(_prep_core_inputs(inputs, c // 2, c % 2))
    res = run_bass_kernel_spmd(nc, in_maps, list(range(NCORES)))
    out = np.empty((B, L, D), np.float32)
    for b in range(B):
        out[b] = res.results[2 * b]["out"].T
    return out


# revision 19
# speedup vs baseline: 1.2884x; 1.1838x over previous
"""Trainium2 Bass kernel for nn_Decoder (ragged expand + 4-layer decoder).

Sharding: 8 cores = 4 pairs. Pair p handles batch b=p (data parallel over B).
Within a pair, Megatron TP-2: wq/wk/wv/w1/w3 column-sharded, wo/w2 row-sharded,
pair AllReduce after attention-out and FFN-out projections.

Device layout: activations live transposed in SBUF as xT[d, l] (partition=d),
tiled [128, n_dtiles, L]. All matmuls bf16 inputs, fp32 PSUM accumulate;
residual stream fp32.
"""

import numpy as np
import ml_dtypes

B, L, D, KB = 4, 1024, 1024, 128
H, HD, NL, FF = 16, 64, 4, 4096
EPS = 1e-5
NCORES = 8
TP = 2
DQ = D // TP      # 512 local q/k/v dims (8 heads)
DF = FF // TP     # 2048 local ffn dims
NH = H // TP      # 8 local heads
CH = 512          # sequence chunk for moving dim
NCH = L // CH     # 2
NDT = D // 128    # 8 d-tiles
NQT = DQ // 128   # 4 local qkv tiles
NFT = DF // 128   # 16 local ffn tiles
NLT = L // 128    # 8 sequence tiles

_CACHE = {}
# debug/bisect toggles (consulted at build time)
OPTS = {"recip_approx": False, "cc_shared": False, "ffn_cc_bf16": True}


def _build(pairs=4):
    import concourse.mybir as mybir
    import concourse.tile as tile
    from concourse import bacc

    f32 = mybir.dt.float32
    bf = mybir.dt.bfloat16
    Alu = mybir.AluOpType
    Act = mybir.ActivationFunctionType

    nc = bacc.Bacc(
        "TRN2", target_bir_lowering=False, debug=False, num_devices=2 * pairs
    )
    sim_mode = pairs == 1  # CoreSim lacks Silu; decompose as x*sigmoid(x)

    # ---- DRAM I/O (per core) ----
    xp_d = nc.dram_tensor("xp", [KB, D], f32, kind="ExternalInput")
    bnd_d = nc.dram_tensor("bnd", [KB, 1], f32, kind="ExternalInput")
    bnds_d = nc.dram_tensor("bnds", [KB, 1], f32, kind="ExternalInput")
    xres_d = nc.dram_tensor("xres", [128, NDT, L], f32, kind="ExternalInput")
    cos_d = nc.dram_tensor("cosr", [128, L], bf, kind="ExternalInput")
    sin_d = nc.dram_tensor("sinr", [128, L], bf, kind="ExternalInput")
    wq_d = nc.dram_tensor("wq", [NL, 128, NDT, DQ], bf, kind="ExternalInput")
    wk_d = nc.dram_tensor("wk", [NL, 128, NDT, DQ], bf, kind="ExternalInput")
    wv_d = nc.dram_tensor("wv", [NL, 128, NDT, DQ], bf, kind="ExternalInput")
    wo_d = nc.dram_tensor("wo", [NL, 128, NDT, D], bf, kind="ExternalInput")
    w1_d = nc.dram_tensor("w1", [NL, 128, NDT, DF], bf, kind="ExternalInput")
    w3_d = nc.dram_tensor("w3", [NL, 128, NDT, DF], bf, kind="ExternalInput")
    w2_d = nc.dram_tensor("w2", [NL, 128, NFT, D], bf, kind="ExternalInput")
    onec_d = nc.dram_tensor("onec", [128, 1], bf, kind="ExternalInput")
    eps_d = nc.dram_tensor("epsc", [1, 1], f32, kind="ExternalInput")
    oner_d = nc.dram_tensor("oner", [1, 128], bf, kind="ExternalInput")
    nwr_d = nc.dram_tensor("nwr", [1, D], bf, kind="ExternalInput")
    rsg_d = nc.dram_tensor("rsg", [128, 128], bf, kind="ExternalInput")
    dmask_d = nc.dram_tensor("dmask", [128, 128], bf, kind="ExternalInput")
    out_d = nc.dram_tensor("out", [D, L], f32, kind="ExternalOutput")

    groups = [[2 * p, 2 * p + 1] for p in range(pairs)]

    with tile.TileContext(nc) as tc:
        with (
            tc.tile_pool(name="persist", bufs=1) as pp,
            tc.tile_pool(name="gbig", bufs=1) as gp,
            tc.tile_pool(name="wres", bufs=1) as wp,
            tc.tile_pool(name="wstream", bufs=3) as ws,
            tc.tile_pool(name="small", bufs=3) as sp,
            tc.tile_pool(name="rows", bufs=2) as rp,
            tc.tile_pool(name="stage", bufs=2) as st,
            tc.tile_pool(name="psA", bufs=2, space="PSUM") as psA,
            tc.tile_pool(name="psB", bufs=2, space="PSUM") as psB,
            tc.tile_pool(name="psC", bufs=2, space="PSUM") as psC,
            tc.tile_pool(name="psD", bufs=1, space="PSUM") as psD,
            tc.tile_pool(name="psE", bufs=1, space="PSUM") as psE,
            tc.tile_pool(name="dram", bufs=2, space="DRAM") as dp,
        ):
            # ---- persistent tiles ----
            xT = pp.tile([128, NDT, L], f32)
            hh = pp.tile([128, NDT, L], bf)
            qT = pp.tile([128, NQT, L], bf)
            kT = pp.tile([128, NQT, L], bf)
            vS = pp.tile([128, NLT, NH * 65], bf)
            oT = pp.tile([128, NQT, L], bf)
            cosr = pp.tile([128, L], bf)
            sinr = pp.tile([128, L], bf)
            onec = pp.tile([128, 1], bf)
            epsc = pp.tile([1, 1], f32)
            oner = pp.tile([1, 128], bf)
            nwr = pp.tile([1, D], bf)
            rsg = pp.tile([128, 128], bf)
            dmask = pp.tile([128, 128], bf)
            bnd = pp.tile([KB, 1], f32)
            bnds = pp.tile([KB, 1], f32)
            # big scratch region (tag-shared): g for FFN, out staging at end
            g = gp.tile([128, NFT, L], bf, tag="big")

            nc.sync.dma_start(cosr[:], cos_d[:])
            nc.sync.dma_start(sinr[:], sin_d[:])
            nc.sync.dma_start(onec[:], onec_d[:])
            nc.sync.dma_start(epsc[:], eps_d[:])
            nc.sync.dma_start(oner[:], oner_d[:])
            nc.sync.dma_start(nwr[:], nwr_d[:])
            nc.sync.dma_start(rsg[:], rsg_d[:])
            nc.sync.dma_start(dmask[:], dmask_d[:])
            nc.sync.dma_start(bnd[:], bnd_d[:])
            nc.sync.dma_start(bnds[:], bnds_d[:])
            nc.sync.dma_start(xT[:], xres_d[:])

            vS_r = vS.rearrange("p t (h e) -> p t h e", e=65)
            nc.gpsimd.memset(vS_r[:, :, :, 64:65], 1.0)

            # ---- ragged expand: xT += one_hot_gather(x_processed) ----
            # setup scratch shares the FFN g region (disjoint lifetimes)
            setup = gp.tile([128, 4, L], f32, tag="big")
            iota, cmp0, gt, xp = (setup[:, i, :] for i in range(4))
            nc.gpsimd.iota(
                iota, pattern=[[1, L]], base=0, channel_multiplier=0,
                allow_small_or_imprecise_dtypes=True,
            )
            nc.vector.tensor_scalar(cmp0, iota, bnd[:], None, Alu.is_ge)
            # gt = (l >= b_j) - (l >= b_{j+1}), computed via two tensor_scalar
            nc.vector.tensor_scalar(gt, iota, bnds[:], None, Alu.is_ge)
            nc.vector.tensor_sub(gt, cmp0, gt)
            nc.sync.dma_start(xp, xp_d[:])
            for dt in range(NDT):
                for ch in range(NCH):
                    cs = slice(ch * CH, (ch + 1) * CH)
                    pool, tg = (psA, "A") if ch else (psC, "C")
                    ps = pool.tile([128, CH], f32, tag=tg, name="pse")
                    nc.tensor.matmul(
                        ps[:], xp[:, dt * 128:(dt + 1) * 128], gt[:, cs],
                        start=True, stop=True,
                    )
                    nc.vector.tensor_add(xT[:, dt, cs], xT[:, dt, cs], ps[:])

            # ---- helpers ----
            def rmsnorm(dest, dest_f32_with_nw=False):
                """h = xT * rsqrt(mean(xT^2)+eps) [* norm_w], written per chunk."""
                for ch in range(NCH):
                    cs = slice(ch * CH, (ch + 1) * CH)
                    ssp = psD.tile([1, CH], f32, tag="D")
                    for dt in range(NDT):
                        sq = sp.tile([128, CH], bf, tag="sc3")
                        nc.scalar.activation(sq[:], xT[:, dt, cs], Act.Square)
                        nc.tensor.matmul(
                            ssp[:], onec[:], sq[:],
                            start=(dt == 0), stop=(dt == NDT - 1),
                        )
                    srow = rp.tile([1, CH], f32, tag="srow")
                    nc.scalar.activation(
                        srow[:], ssp[:], Act.Sqrt, bias=epsc[:], scale=1.0 / D
                    )
                    rf32 = rp.tile([1, CH], f32, tag="rf32")
                    if OPTS["recip_approx"]:
                        nc.vector.reciprocal_approx_fast(rf32[:], srow[:])
                    else:
                        nc.vector.reciprocal(rf32[:], srow[:])
                    rrow = rp.tile([1, CH], bf, tag="rbf")
                    nc.scalar.activation(rrow[:], rf32[:], Act.Copy)
                    if not dest_f32_with_nw:
                        bps = psE.tile([128, CH], f32, tag="E")
                        nc.tensor.matmul(
                            bps[:], oner[:], rrow[:], start=True, stop=True
                        )
                        for dt in range(NDT):
                            nc.vector.tensor_mul(
                                dest[:, dt, cs], xT[:, dt, cs], bps[:]
                            )
                    else:
                        for dt in range(NDT):
                            bps = psE.tile([128, CH], f32, tag="E")
                            nc.tensor.matmul(
                                bps[:], nwr[:, dt * 128:(dt + 1) * 128],
                                rrow[:], start=True, stop=True,
                            )
                            nc.vector.tensor_mul(
                                dest[:, dt, cs], xT[:, dt, cs], bps[:]
                            )

            # ==================== layers ====================
            for ly in range(NL):
                # ---- attention ----
                rmsnorm(hh)
                wq = wp.tile([128, NDT, DQ], bf, tag="wq")
                wk = wp.tile([128, NDT, DQ], bf, tag="wk")
                wv = wp.tile([128, NDT, DQ], bf, tag="wv")
                wo = wp.tile([128, NDT, D], bf, tag="wo")
                nc.sync.dma_start(wq[:], wq_d[ly])
                nc.sync.dma_start(wk[:], wk_d[ly])
                nc.sync.dma_start(wv[:], wv_d[ly])
                nc.sync.dma_start(wo[:], wo_d[ly])

                # q/k projections + rope
                for wt, dest in ((wq, qT), (wk, kT)):
                    for ot in range(NQT):
                        for ch in range(NCH):
                            cs = slice(ch * CH, (ch + 1) * CH)
                            pool, tg = ((psA, "A") if (2 * ot + ch) % 2
                                        else (psC, "C"))
                            ps = pool.tile([128, CH], f32, tag=tg, name="ps")
                            for dt in range(NDT):
                                nc.tensor.matmul(
                                    ps[:],
                                    wt[:, dt, ot * 128:(ot + 1) * 128],
                                    hh[:, dt, cs],
                                    start=(dt == 0), stop=(dt == NDT - 1),
                                )
                            qsb = sp.tile([128, CH], bf, tag="sc1")
                            nc.scalar.activation(qsb[:], ps[:], Act.Copy)
                            t_t = sp.tile([128, CH], bf, tag="sc2")
                            u_t = sp.tile([128, CH], bf, tag="sc3")
                            nc.vector.tensor_mul(t_t[:], qsb[:], cosr[:, cs])
                            nc.vector.tensor_mul(u_t[:], qsb[:], sinr[:, cs])
                            # rotate-half with sign via permutation matmul
                            us = psB.tile([128, CH], f32, tag="B")
                            nc.tensor.matmul(
                                us[:], rsg[:], u_t[:], start=True, stop=True
                            )
                            nc.vector.tensor_add(
                                dest[:, ot, cs], t_t[:], us[:]
                            )

                # v projection, non-transposed [l, dv], with ones column kept
                for lt in range(NLT):
                    ls = slice(lt * 128, (lt + 1) * 128)
                    pool, tg = (psA, "A") if lt % 2 else (psC, "C")
                    ps = pool.tile([128, DQ], f32, tag=tg, name="psv")
                    for dt in range(NDT):
                        nc.tensor.matmul(
                            ps[:], hh[:, dt, ls], wv[:, dt, :],
                            start=(dt == 0), stop=(dt == NDT - 1),
                        )
                    nc.scalar.activation(
                        vS_r[:, lt, :, 0:64],
                        ps[:].rearrange("p (h e) -> p h e", e=64),
                        Act.Copy,
                    )

                # attention per (head, chunk): sT scores -> exp/mask -> o accum
                for h8 in range(NH):
                    pb = h8 // 2
                    po = (h8 % 2) * 64
                    for ch in range(NCH):
                        cs = slice(ch * CH, (ch + 1) * CH)
                        nlts = 4 if ch == 0 else 8
                        pot = psC.tile([65, CH], f32, tag="C")
                        for lt in range(nlts):
                            pool, tg = (psA, "A") if lt % 2 else (psB, "B")
                            d0 = 128 * lt - CH * ch
                            v0 = max(0, d0)
                            ss = pool.tile([128, CH], f32, tag=tg, name="ss")
                            nc.tensor.matmul(
                                ss[:, v0:CH],
                                kT[po:po + 64, pb, lt * 128:(lt + 1) * 128],
                                qT[po:po + 64, pb, ch * CH + v0:(ch + 1) * CH],
                                start=True, stop=True,
                            )
                            at = sp.tile([128, CH], bf, tag="sc1")
                            nc.scalar.activation(
                                at[:, v0:CH], ss[:, v0:CH], Act.Exp, scale=0.125
                            )
                            if 0 <= d0 < CH:
                                nc.gpsimd.affine_select(
                                    at[:, d0:d0 + 128], at[:, d0:d0 + 128],
                                    pattern=[[1, 128]],
                                    compare_op=Alu.is_ge,
                                    fill=0.0, base=0, channel_multiplier=-1,
                                )
                            # masked cols [0:v0) are excluded from the o-accum
                            # except on the last tile, which must close the
                            # accumulation group full-range (zero the strip)
                            last = lt == nlts - 1
                            if last and v0 > 0:
                                nc.gpsimd.memset(at[:, 0:v0], 0.0)
                            o0 = 0 if last else v0
                            nc.tensor.matmul(
                                pot[:, o0:CH],
                                vS[:, lt, h8 * 65:(h8 + 1) * 65], at[:, o0:CH],
                                start=(lt == 0), stop=last,
                                skip_group_check=(o0 > 0),
                            )
                        rcf = rp.tile([1, CH], f32, tag="rf32")
                        if OPTS["recip_approx"]:
                            nc.vector.reciprocal_approx_fast(rcf[:], pot[64:65, :])
                        else:
                            nc.vector.reciprocal(rcf[:], pot[64:65, :])
                        rc = rp.tile([1, CH], bf, tag="rbf")
                        nc.scalar.activation(rc[:], rcf[:], Act.Copy)
                        bps = psE.tile([128, CH], f32, tag="E")
                        nc.tensor.matmul(
                            bps[0:64, :], oner[:, 0:64], rc[:],
                            start=True, stop=True,
                        )
                        bsb = sp.tile([64, CH], bf, tag="sc2")
                        nc.scalar.activation(bsb[:], bps[0:64, :], Act.Copy)
                        nc.vector.tensor_mul(
                            oT[po:po + 64, pb, cs], pot[0:64, :], bsb[:]
                        )

                # allgather local oT halves, then full-width wo + residual
                pools8 = (psA, psA, psB, psB, psC, psC, psD, psE)
                tags8 = ("A", "A", "B", "B", "C", "C", "D", "E")
                for ch in range(NCH):
                    cs = slice(ch * CH, (ch + 1) * CH)
                    adsp = "Shared" if OPTS["cc_shared"] else "Local"
                    agi = dp.tile([128, NQT, CH], bf, tag=f"ag{ly}i",
                                  addr_space=adsp)
                    ago = dp.tile([2, 128, NQT, CH], bf, tag=f"ag{ly}o",
                                  addr_space=adsp)
                    nc.gpsimd.dma_start(agi[:], oT[:, :, cs])
                    nc.gpsimd.collective_compute(
                        "AllGather", Alu.bypass, replica_groups=groups,
                        ins=[agi.opt()], outs=[ago.opt()],
                    )
                    pso = [
                        pools8[ot].tile(
                            [128, CH], f32, tag=tags8[ot], name=f"wops_{ot}"
                        )
                        for ot in range(NDT)
                    ]
                    for kt in range(NDT):
                        og = ws.tile([128, CH], bf, tag="og")
                        nc.gpsimd.dma_start(og[:], ago[kt // NQT, :, kt % NQT, :])
                        for ot in range(NDT):
                            nc.tensor.matmul(
                                pso[ot][:],
                                wo[:, kt, ot * 128:(ot + 1) * 128], og[:],
                                start=(kt == 0), stop=(kt == NDT - 1),
                            )
                    for ot in range(NDT):
                        nc.vector.tensor_add(
                            xT[:, ot, cs], xT[:, ot, cs], pso[ot][:]
                        )

                # ---- FFN ----
                rmsnorm(hh)
                for ch in range(NCH):
                    cs = slice(ch * CH, (ch + 1) * CH)
                    # w1 -> silu -> g
                    for ftb in range(4):
                        pss = []
                        for j in range(4):
                            tagp = ("A", "A", "B", "B")[j]
                            pool = (psA, psA, psB, psB)[j]
                            pss.append(pool.tile(
                                [128, CH], f32, tag=tagp, name=f"ps1_{j}"
                            ))
                        for dt in range(NDT):
                            w1s = ws.tile([128, 512], bf, tag="w1s")
                            nc.sync.dma_start(
                                w1s[:], w1_d[ly, :, dt, ftb * 512:(ftb + 1) * 512]
                            )
                            for j in range(4):
                                nc.tensor.matmul(
                                    pss[j][:],
                                    w1s[:, j * 128:(j + 1) * 128],
                                    hh[:, dt, cs],
                                    start=(dt == 0), stop=(dt == NDT - 1),
                                )
                        for j in range(4):
                            if sim_mode:
                                sgt = sp.tile([128, CH], bf, tag="sc2",
                                              name="sgt")
                                nc.scalar.activation(
                                    sgt[:], pss[j][:], Act.Sigmoid
                                )
                                nc.vector.tensor_mul(
                                    g[:, ftb * 4 + j, cs], sgt[:], pss[j][:]
                                )
                            else:
                                nc.scalar.activation(
                                    g[:, ftb * 4 + j, cs], pss[j][:], Act.Silu
                                )
                    # w3 -> u = g * w3 (in place in g)
                    for ftb in range(4):
                        pss = []
                        for j in range(4):
                            tagp = ("A", "A", "B", "B")[j]
                            pool = (psA, psA, psB, psB)[j]
                            pss.append(pool.tile(
                                [128, CH], f32, tag=tagp, name=f"ps3_{j}"
                            ))
                        for dt in range(NDT):
                            w3s = ws.tile([128, 512], bf, tag="w3s")
                            nc.sync.dma_start(
                                w3s[:], w3_d[ly, :, dt, ftb * 512:(ftb + 1) * 512]
                            )
                            for j in range(4):
                                nc.tensor.matmul(
                                    pss[j][:],
                                    w3s[:, j * 128:(j + 1) * 128],
                                    hh[:, dt, cs],
                                    start=(dt == 0), stop=(dt == NDT - 1),
                                )
                        for j in range(4):
                            ft = ftb * 4 + j
                            nc.vector.tensor_mul(
                                g[:, ft, cs], g[:, ft, cs], pss[j][:]
                            )
                    # w2: 8 concurrent psums over all banks
                    pso = [
                        pools8[ot].tile(
                            [128, CH], f32, tag=tags8[ot], name=f"pso_{ot}"
                        )
                        for ot in range(NDT)
                    ]
                    for ft in range(NFT):
                        w2s = ws.tile([128, D], bf, tag="w2s")
                        nc.sync.dma_start(w2s[:], w2_d[ly, :, ft, :])
                        for ot in range(NDT):
                            nc.tensor.matmul(
                                pso[ot][:],
                                w2s[:, ot * 128:(ot + 1) * 128],
                                g[:, ft, cs],
                                start=(ft == 0), stop=(ft == NFT - 1),
                            )
                    # allreduce this chunk + residual
                    ccdt = bf if OPTS["ffn_cc_bf16"] else f32
                    adsp = "Shared" if OPTS["cc_shared"] else "Local"
                    inb = dp.tile([128, NDT, CH], ccdt, tag=f"ff{ly}i",
                                  addr_space=adsp)
                    outb = dp.tile([128, NDT, CH], ccdt, tag=f"ff{ly}o",
                                   addr_space=adsp)
                    for ot in range(NDT):
                        stg = st.tile([128, CH], ccdt, tag="stg")
                        nc.scalar.activation(stg[:], pso[ot][:], Act.Copy)
                        nc.gpsimd.dma_start(inb[:, ot, :], stg[:])
                    nc.gpsimd.collective_compute(
                        "AllReduce", Alu.add, replica_groups=groups,
                        ins=[inb.opt()], outs=[outb.opt()],
                    )
                    for ot in range(NDT):
                        ret = st.tile([128, CH], ccdt, tag="ret")
                        nc.gpsimd.dma_start(ret[:], outb[:, ot, :])
                        nc.vector.tensor_add(
                            xT[:, ot, cs], xT[:, ot, cs], ret[:]
                        )

            # ---- final norm (* norm_w) and output ----
            outsb = gp.tile([128, NDT, L], f32, tag="big")
            rmsnorm(outsb, dest_f32_with_nw=True)
            nc.sync.dma_start(out_d[:].rearrange("(dt p) l -> p dt l", p=128), outsb[:])

    nc.finalize()
    return nc


def _get_nc(pairs=4):
    if pairs not in _CACHE:
        _CACHE[pairs] = _build(pairs)
    return _CACHE[pairs]


def _rsign_const():
    # usigned = R2 @ u : usigned[0:32] = -u[32:64], usigned[32:64] = u[0:32]
    # per 64-block (two heads per 128-partition tile). Pass lhsT = R2.T.
    rh = np.zeros((64, 64), np.float32)
    rh[np.arange(32), np.arange(32) + 32] = -1.0
    rh[np.arange(32) + 32, np.arange(32)] = 1.0
    r2 = np.zeros((128, 128), np.float32)
    r2[0:64, 0:64] = rh
    r2[64:128, 64:128] = rh
    return np.ascontiguousarray(r2.T).astype(ml_dtypes.bfloat16)


def _dmask_const():
    # additive causal mask for a diagonal 128x128 block of sT [l', l]:
    # keep (0.0) where l' <= l i.e. p <= f, else -1e30
    p = np.arange(128)
    m = np.where(p[:, None] <= p[None, :], 0.0, -1e30).astype(np.float32)
    return m.astype(ml_dtypes.bfloat16)


def _prep_core_inputs(inputs, b, t):
    """Host-side shard/layout prep for core (pair b, tp half t)."""
    f32 = np.float32
    bf = ml_dtypes.bfloat16
    x_processed = np.asarray(inputs["x_processed"], f32)
    boundaries = np.asarray(inputs["boundaries"], np.int32)
    x_residual = np.asarray(inputs["x_residual"], f32)
    cos = np.asarray(inputs["cos"], f32)
    sin = np.asarray(inputs["sin"], f32)
    wq = np.asarray(inputs["wq"], f32)
    wk = np.asarray(inputs["wk"], f32)
    wv = np.asarray(inputs["wv"], f32)
    wo = np.asarray(inputs["wo"], f32)
    w1 = np.asarray(inputs["w1"], f32)
    w2 = np.asarray(inputs["w2"], f32)
    w3 = np.asarray(inputs["w3"], f32)
    attn_norm_w = np.asarray(inputs["attn_norm_w"], f32)
    ffn_norm_w = np.asarray(inputs["ffn_norm_w"], f32)
    norm_w = np.asarray(inputs["norm_w"], f32)

    bnd = boundaries[b].astype(f32).copy()
    bnd[0] = 0.0  # match searchsorted-then-clip(>=0) semantics
    bnds = np.concatenate([bnd[1:], [np.float32(2 * L)]])

    def dtile(w, ncols):  # [D, ncols] -> [128, D//128, ncols]
        return np.ascontiguousarray(
            w.reshape(-1, 128, ncols).transpose(1, 0, 2)
        )

    qs = slice(t * DQ, (t + 1) * DQ)
    fs = slice(t * DF, (t + 1) * DF)
    wq_s = np.stack([
        dtile((attn_norm_w[l][:, None] * wq[l])[:, qs].astype(bf), DQ)
        for l in range(NL)
    ])
    wk_s = np.stack([
        dtile((attn_norm_w[l][:, None] * wk[l])[:, qs].astype(bf), DQ)
        for l in range(NL)
    ])
    wv_s = np.stack([
        dtile((attn_norm_w[l][:, None] * wv[l])[:, qs].astype(bf), DQ)
        for l in range(NL)
    ])
    wo_s = np.stack([dtile(wo[l].astype(bf), D) for l in range(NL)])
    w1_s = np.stack([
        dtile((ffn_norm_w[l][:, None] * w1[l])[:, fs].astype(bf), DF)
        for l in range(NL)
    ])
    w3_s = np.stack([
        dtile((ffn_norm_w[l][:, None] * w3[l])[:, fs].astype(bf), DF)
        for l in range(NL)
    ])
    w2_s = np.stack([dtile(w2[l][fs, :].astype(bf), D) for l in range(NL)])

    cosT = cos.T.astype(bf)  # [HD, L]
    sinT = sin.T.astype(bf)
    cos_rep = np.concatenate([cosT, cosT], axis=0)  # [128, L]
    sin_rep = np.concatenate([sinT, sinT], axis=0)

    xres_t = np.ascontiguousarray(
        x_residual[b].T.reshape(NDT, 128, L).transpose(1, 0, 2)
    )

    return {
        "xp": np.ascontiguousarray(x_processed[b]),
        "bnd": bnd[:, None],
        "bnds": bnds[:, None],
        "xres": xres_t,
        "cosr": np.ascontiguousarray(cos_rep),
        "sinr": np.ascontiguousarray(sin_rep),
        "wq": wq_s, "wk": wk_s, "wv": wv_s, "wo": wo_s,
        "w1": w1_s, "w3": w3_s, "w2": w2_s,
        "onec": np.ones((128, 1), bf),
        "rsg": _rsign_const(),
        "dmask": _dmask_const(),
        "epsc": np.full((1, 1), EPS, f32),
        "oner": np.ones((1, 128), bf),
        "nwr": norm_w[None, :].astype(bf),
    }


def kernel(**inputs) -> np.ndarray:
    from concourse.bass_utils import run_bass_kernel_spmd

    nc = _get_nc(4)
    in_maps = []
    for c in range(NCORES):
        in_maps.append(_prep_core_inputs(inputs, c // 2, c % 2))
    res = run_bass_kernel_spmd(nc, in_maps, list(range(NCORES)))
    out = np.empty((B, L, D), np.float32)
    for b in range(B):
        out[b] = res.results[2 * b]["out"].T
    return out
